# revision 1
# baseline (speedup 1.0000x reference)
"""3-layer GAT + linear head on 8 Trainium2 NeuronCores.

Strategy: destination-sharded edge parallelism.
 - Host relabels nodes by in-degree (desc) and deals 128-node chunks
   round-robin to the 8 cores, so per-core edge counts and per-chunk max
   degrees are balanced. Each core owns 6272 destination slots (49 chunks).
 - Per layer, every core computes the full node transform T[n] = [h@W | h@W@a_s]
   (replicated, feature-major matmul), writes it to a DRAM table, then for its
   own chunks indirect-DMA-gathers source rows per edge slot (dst-major padded
   layout: partition = destination, free = padded in-edge slot).
 - Softmax over in-edges is exact (free-dim reduce_max / reduce_sum);
   aggregation accumulates w_k * G_k into PSUM via identity-stationary matmuls.
 - Layers exchange hidden states with an 8-rank AllGather; the final linear
   layer is purely shard-local.
"""
import sys
sys.path.insert(0, '/opt/trn_rl_repo')
import numpy as np
import ml_dtypes

N = 50000
E = 800000
D = 128
NCORES = 8
NP = 50176          # N padded to 392 chunks of 128
SHARD = NP // NCORES  # 6272
NCHUNK = SHARD // 128  # 49
NTILES = NP // 128     # 392
NEG_SLOPE = 0.2

_cache = {}


def _legalize_single_wait(nc, mybir):
    ctr = 0
    for fn in nc.m.functions:
        for bb in fn.blocks:
            insts = bb.instructions
            out = []
            changed = False
            for inst in insts:
                si = getattr(inst, 'sync_info', None) if hasattr(inst, 'sync_info') else None
                waits = list(si.on_wait) if si and si.on_wait else []
                if len(waits) > 1:
                    eng = inst.engine
                    for w in waits[:-1]:
                        ctr += 1
                        nop = mybir.InstNoOp(name=f"legwait-{ctr}", ins=[], outs=[])
                        nop.engine = eng
                        nop.sync_info = mybir.SyncInfo(on_wait=[w], on_update=[])
                        out.append(nop)
                    inst.sync_info = mybir.SyncInfo(
                        on_wait=waits[-1:], on_update=list(si.on_update or []))
                    changed = True
                out.append(inst)
            if changed:
                bb.instructions = out


def _build_nc(KBAR):
    import concourse.bass as bass
    import concourse.mybir as mybir
    from concourse.tile import TileContext

    SK = int(sum(KBAR))
    f32 = mybir.dt.float32
    bf16 = mybir.dt.bfloat16

    nc = bass.Bass()
    # inputs (per-core where noted)
    embT = nc.dram_tensor("embT", [128, NP], bf16, kind="ExternalInput")
    h0T_sh = nc.dram_tensor("h0T_sh", [128, SHARD], bf16, kind="ExternalInput")
    srcidx = nc.dram_tensor("srcidx", [128, SK], mybir.dt.int32, kind="ExternalInput")  # per-core
    maskin = nc.dram_tensor("maskin", [128, SK], f32, kind="ExternalInput")  # per-core
    Wexts, wads, b_reps, p_cols = [], [], [], []
    for l in (1, 2, 3):
        Wexts.append(nc.dram_tensor(f"Wext{l}", [128, 129], bf16, kind="ExternalInput"))
        wads.append(nc.dram_tensor(f"wad{l}", [128, 1], bf16, kind="ExternalInput"))
        b_reps.append(nc.dram_tensor(f"brep{l}", [128, 128], f32, kind="ExternalInput"))
        p_cols.append(nc.dram_tensor(f"pcol{l}", [128, 1], f32, kind="ExternalInput"))
    Wo = nc.dram_tensor("Wo", [128, 128], bf16, kind="ExternalInput")
    bo_rep = nc.dram_tensor("borep", [128, 128], f32, kind="ExternalInput")
    ident = nc.dram_tensor("ident", [128, 128], f32, kind="ExternalInput")
    out_sh = nc.dram_tensor("out_sh", [SHARD, 128], f32, kind="ExternalOutput")
    # internals
    T_dram = nc.dram_tensor("T_dram", [NP, 129], f32, kind="Internal")
    h_rm = nc.dram_tensor("h_rm", [SHARD, 128], bf16, kind="Internal")
    h_full = nc.dram_tensor("h_full", [NP, 128], bf16, kind="Internal", addr_space="Shared")

    EV = 3  # tiles per T-evac group (3*130=390 f32 <= 512: single PSUM bank)

    with TileContext(nc) as tc:
        with (
            tc.tile_pool(name="consts", bufs=1) as cpool,
            tc.tile_pool(name="hT", bufs=6) as hpool,
            tc.tile_pool(name="tstage", bufs=3) as tspool,
            tc.tile_pool(name="gbuf", bufs=3) as gpool,
            tc.tile_pool(name="sca", bufs=4) as spool,
            tc.tile_pool(name="ev", bufs=3) as epool,
            tc.tile_pool(name="psg", bufs=2, space="PSUM") as psg,
            tc.tile_pool(name="psa", bufs=4, space="PSUM") as psa,
            tc.tile_pool(name="pse", bufs=2, space="PSUM") as pse,
        ):
            ident_sb = cpool.tile([128, 128], f32)
            nc.sync.dma_start(ident_sb[:], ident[:])
            Wext_sb, wad_sb, brep_sb, pcol_sb = [], [], [], []
            for l in range(3):
                t = cpool.tile([128, 129], bf16, tag=f"we{l}")
                nc.sync.dma_start(t[:], Wexts[l][:])
                Wext_sb.append(t)
                t = cpool.tile([128, 1], bf16, tag=f"wa{l}")
                nc.sync.dma_start(t[:], wads[l][:])
                wad_sb.append(t)
                t = cpool.tile([128, 128], f32, tag=f"br{l}")
                nc.sync.dma_start(t[:], b_reps[l][:])
                brep_sb.append(t)
                t = cpool.tile([128, 1], f32, tag=f"pc{l}")
                nc.sync.dma_start(t[:], p_cols[l][:])
                pcol_sb.append(t)
            Wo_sb = cpool.tile([128, 128], bf16)
            nc.sync.dma_start(Wo_sb[:], Wo[:])
            borep_sb = cpool.tile([128, 128], f32)
            nc.sync.dma_start(borep_sb[:], bo_rep[:])
            srcidx_sb = cpool.tile([128, SK], mybir.dt.int32)
            nc.sync.dma_start(srcidx_sb[:], srcidx[:])
            mask_sb = cpool.tile([128, SK], f32)
            nc.sync.dma_start(mask_sb[:], maskin[:])

            koff = np.zeros(NCHUNK + 1, np.int64)
            for j in range(NCHUNK):
                koff[j + 1] = koff[j] + KBAR[j]

            for layer in range(3):
                # ---- per-node transform: T[n] = [h@W | h@W@a_s] for all NP nodes
                for g0 in range(0, NTILES, EV):
                    gn = min(EV, NTILES - g0)
                    pg = psg.tile([128, EV * 130], f32, tag="pg")
                    for q in range(gn):
                        t0 = g0 + q
                        hT = hpool.tile([128, 128], bf16, tag="hT")
                        if layer == 0:
                            nc.sync.dma_start(hT[:], embT[:, t0 * 128:(t0 + 1) * 128])
                        else:
                            nc.sync.dma_start_transpose(hT[:], h_full[t0 * 128:(t0 + 1) * 128, :])
                        nc.tensor.matmul(pg[:, q * 130:q * 130 + 129], lhsT=hT[:],
                                         rhs=Wext_sb[layer][:], start=True, stop=True)
                    ts = tspool.tile([128, EV * 129], f32, tag="ts")
                    pg3 = pg[:].rearrange("p (q e) -> p q e", e=130)
                    ts3 = ts[:].rearrange("p (q e) -> p q e", e=129)
                    nc.scalar.activation(ts3[:, 0:gn, :], pg3[:, 0:gn, 0:129],
                                         mybir.ActivationFunctionType.Copy)
                    # T rows for tiles g0..g0+gn
                    nc.sync.dma_start(
                        T_dram[g0 * 128:(g0 + gn) * 128, :].rearrange(
                            "(q p) e -> p q e", p=128),
                        ts3[:, 0:gn, :])

                # ---- ed for own shard (49 tiles)
                ed_sb = spool.tile([128, NCHUNK], f32, tag="ed")
                for j in range(NCHUNK):
                    hT = hpool.tile([128, 128], bf16, tag="hT")
                    if layer == 0:
                        nc.sync.dma_start(hT[:], h0T_sh[:, j * 128:(j + 1) * 128])
                    else:
                        nc.sync.dma_start_transpose(hT[:], h_rm[j * 128:(j + 1) * 128, :])
                    pe = pse.tile([128, 1], f32, tag="pe")
                    nc.tensor.matmul(pe[:], lhsT=hT[:], rhs=wad_sb[layer][:],
                                     start=True, stop=True)
                    nc.scalar.activation(ed_sb[:, j:j + 1], pe[:],
                                         mybir.ActivationFunctionType.Copy)

                # ---- per-chunk gather + softmax + aggregate
                for j in range(NCHUNK):
                    K = int(KBAR[j])
                    o0 = int(koff[j])
                    Gc = gpool.tile([128, K * 129], f32, tag="Gc")
                    G3 = Gc[:].rearrange("p (k e) -> p k e", e=129)
                    for k in range(K):
                        nc.gpsimd.indirect_dma_start(
                            out=G3[:, k, :],
                            out_offset=None,
                            in_=T_dram[:],
                            in_offset=bass.IndirectOffsetOnAxis(
                                ap=srcidx_sb[:, o0 + k:o0 + k + 1], axis=0),
                        )
                    # scalars
                    tE = spool.tile([128, K], f32, tag="tE")
                    nc.vector.tensor_scalar(out=tE[:], in0=G3[:, :, 128],
                                            scalar1=ed_sb[:, j:j + 1], scalar2=None,
                                            op0=mybir.AluOpType.add)
                    tL = spool.tile([128, K], f32, tag="tL")
                    nc.scalar.activation(tL[:], tE[:], mybir.ActivationFunctionType.Lrelu,
                                         alpha=NEG_SLOPE)
                    mx = spool.tile([128, 1], f32, tag="mx")
                    nc.vector.tensor_reduce(mx[:], tL[:], axis=mybir.AxisListType.X,
                                            op=mybir.AluOpType.max)
                    ngm = spool.tile([128, 1], f32, tag="ngm")
                    nc.vector.tensor_scalar(out=ngm[:], in0=mx[:], scalar1=-1.0,
                                            scalar2=None, op0=mybir.AluOpType.mult)
                    wE = spool.tile([128, K], f32, tag="wE")
                    nc.scalar.activation(wE[:], tL[:], mybir.ActivationFunctionType.Exp,
                                         bias=ngm[:, 0:1], scale=1.0)
                    w2 = spool.tile([128, K], f32, tag="w2")
                    nc.vector.tensor_tensor(out=w2[:], in0=wE[:],
                                            in1=mask_sb[:, o0:o0 + K],
                                            op=mybir.AluOpType.mult)
                    zz = spool.tile([128, 1], f32, tag="zz")
                    nc.vector.tensor_reduce(zz[:], w2[:], axis=mybir.AxisListType.X,
                                            op=mybir.AluOpType.add)
                    zc = spool.tile([128, 1], f32, tag="zc")
                    nc.vector.tensor_scalar(out=zc[:], in0=zz[:], scalar1=1e-30,
                                            scalar2=None, op0=mybir.AluOpType.max)
                    zi = spool.tile([128, 1], f32, tag="zi")
                    nc.vector.reciprocal(zi[:], zc[:])
                    pa = psa.tile([128, 128], f32, tag="pa")
                    for k in range(K):
                        Gs = gpool.tile([128, 128], f32, tag="Gs")
                        nc.vector.tensor_scalar(out=Gs[:], in0=G3[:, k, 0:128],
                                                scalar1=w2[:, k:k + 1], scalar2=None,
                                                op0=mybir.AluOpType.mult)
                        nc.tensor.matmul(pa[:], lhsT=ident_sb[:], rhs=Gs[:],
                                         start=(k == 0), stop=(k == K - 1))
                    o1 = epool.tile([128, 128], f32, tag="o1")
                    nc.scalar.activation(o1[:], pa[:], mybir.ActivationFunctionType.Copy,
                                         scale=zi[:, 0:1])
                    h1 = epool.tile([128, 128], f32, tag="h1")
                    nc.vector.tensor_tensor(out=h1[:], in0=o1[:], in1=brep_sb[layer][:],
                                            op=mybir.AluOpType.add)
                    tp = epool.tile([128, 128], f32, tag="tp")
                    nc.vector.tensor_scalar(out=tp[:], in0=h1[:],
                                            scalar1=pcol_sb[layer][:, 0:1], scalar2=None,
                                            op0=mybir.AluOpType.mult)
                    hn = epool.tile([128, 128], bf16, tag="hn")
                    nc.vector.tensor_tensor(out=hn[:], in0=h1[:], in1=tp[:],
                                            op=mybir.AluOpType.max)
                    nc.sync.dma_start(h_rm[j * 128:(j + 1) * 128, :], hn[:])

                if layer < 2:
                    nc.gpsimd.collective_compute(
                        "AllGather", mybir.AluOpType.bypass,
                        ins=[h_rm[:]], outs=[h_full[:]],
                        replica_groups=[list(range(NCORES))],
                    )

            # ---- final linear: out = h3 @ Wo + bo  (shard-local)
            for j in range(NCHUNK):
                hT = hpool.tile([128, 128], bf16, tag="hT")
                nc.sync.dma_start_transpose(hT[:], h_rm[j * 128:(j + 1) * 128, :])
                po = psa.tile([128, 128], f32, tag="pa")
                nc.tensor.matmul(po[:], lhsT=hT[:], rhs=Wo_sb[:], start=True, stop=True)
                oo = epool.tile([128, 128], f32, tag="oo")
                nc.scalar.activation(oo[:], po[:], mybir.ActivationFunctionType.Copy)
                o2 = epool.tile([128, 128], f32, tag="o2")
                nc.vector.tensor_tensor(out=o2[:], in0=oo[:], in1=borep_sb[:],
                                        op=mybir.AluOpType.add)
                nc.sync.dma_start(out_sh[j * 128:(j + 1) * 128, :], o2[:])

    _legalize_single_wait(nc, mybir)
    return nc


class _Runner:
    def __init__(self, nc, in_maps, n_cores):
        import jax
        import concourse.mybir as mybir
        from concourse.bass2jax import (_bass_exec_p, partition_id_tensor,
                                        install_neuronx_cc_hook)
        from jax.sharding import Mesh, PartitionSpec
        from jax.experimental.shard_map import shard_map
        install_neuronx_cc_hook()
        self.jax = jax
        self.n_cores = n_cores
        in_names, out_names, out_avals, zero_outs = [], [], [], []
        partition_name = nc.partition_id_tensor.name if nc.partition_id_tensor else None
        import concourse.mybir as mybir
        for alloc in nc.m.functions[0].allocations:
            if not isinstance(alloc, mybir.MemoryLocationSet):
                continue
            name = alloc.memorylocations[0].name
            if alloc.kind == "ExternalInput":
                if name != partition_name:
                    in_names.append(name)
            elif alloc.kind == "ExternalOutput":
                shape = tuple(alloc.tensor_shape)
                dtype = mybir.dt.np(alloc.dtype)
                out_names.append(name)
                out_avals.append(jax.core.ShapedArray(shape, dtype))
                zero_outs.append(np.zeros(shape, dtype))
        n_params = len(in_names)
        self.out_names, self.out_avals = out_names, out_avals
        all_in = list(in_names) + list(out_names)
        if partition_name is not None:
            all_in.append(partition_name)

        def _body(*args):
            operands = list(args)
            if partition_name is not None:
                operands.append(partition_id_tensor())
            outs = _bass_exec_p.bind(
                *operands, out_avals=tuple(out_avals), in_names=tuple(all_in),
                out_names=tuple(out_names), lowering_input_output_aliases=(),
                sim_require_finite=False, sim_require_nnan=False, nc=nc)
            return tuple(outs)

        devices = jax.devices()[:n_cores]
        mesh = Mesh(np.asarray(devices), ("core",))
        self.fn = jax.jit(
            shard_map(_body, mesh=mesh,
                      in_specs=(PartitionSpec("core"),) * (n_params + len(out_names)),
                      out_specs=(PartitionSpec("core"),) * len(out_names),
                      check_rep=False),
            keep_unused=True)
        per_core = [[np.asarray(m[nm]) for nm in in_names] for m in in_maps]
        concat_in = [np.concatenate([per_core[c][i] for c in range(n_cores)], axis=0)
                     for i in range(n_params)]
        concat_zeros = [np.zeros((n_cores * z.shape[0], *z.shape[1:]), z.dtype)
                        for z in zero_outs]
        sh = jax.sharding.NamedSharding(mesh, PartitionSpec("core"))
        self.dev_args = [jax.device_put(a, sh) for a in concat_in + concat_zeros]

    def run_raw(self):
        return self.fn(*self.dev_args)

    def results(self):
        outs = self.run_raw()
        self.jax.block_until_ready(outs)
        return [
            {nm: np.asarray(outs[i]).reshape(self.n_cores, *self.out_avals[i].shape)[c]
             for i, nm in enumerate(self.out_names)}
            for c in range(self.n_cores)]


def _prepare(x, edge_index, emb, weights):
    """Host-side: relabel, chunk, schedule, build per-core inputs."""
    (W1, as1, ad1, b1, p1, W2, as2, ad2, b2, p2,
     W3, as3, ad3, b3, p3, Wo, bo) = weights
    h0 = np.asarray(emb)[np.asarray(x)]  # [N, D] f32
    src = np.asarray(edge_index[0], np.int64)
    dst = np.asarray(edge_index[1], np.int64)
    src = np.concatenate([src, np.arange(N, dtype=np.int64)])
    dst = np.concatenate([dst, np.arange(N, dtype=np.int64)])

    deg = np.bincount(dst, minlength=NP)  # pad nodes deg 0
    order = np.argsort(-deg, kind="stable")  # [NP]
    pos = np.empty(NP, np.int64)
    # chunk rank r -> core r%8, local j=r//8; pos = core*SHARD + j*128 + i
    for r in range(NTILES):
        nodes = order[r * 128:(r + 1) * 128]
        core, j = r % NCORES, r // NCORES
        pos[nodes] = core * SHARD + j * 128 + np.arange(128)

    srcp = pos[src]
    dstp = pos[dst]

    # group edges by dst position
    o = np.argsort(dstp, kind="stable")
    dst_sorted = dstp[o]
    src_sorted = srcp[o]
    starts = np.searchsorted(dst_sorted, np.arange(NP))
    ends = np.searchsorted(dst_sorted, np.arange(NP) + 1)
    degs_pos = ends - starts  # degree by position

    # KBAR[j] = max degree among all cores' chunks with local index j
    dp = degs_pos.reshape(NCORES, NCHUNK, 128)
    KBAR = dp.max(axis=(0, 2)).astype(np.int64)  # [NCHUNK]
    KBAR = np.maximum(KBAR, 1)
    SK = int(KBAR.sum())

    srcidx = np.zeros((NCORES, 128, SK), np.int32)
    mask = np.zeros((NCORES, 128, SK), np.float32)
    koff = np.concatenate([[0], np.cumsum(KBAR)])
    for c in range(NCORES):
        for j in range(NCHUNK):
            base = c * SHARD + j * 128
            K = int(KBAR[j])
            for p in range(128):
                s, e = starts[base + p], ends[base + p]
                d = e - s
                if d:
                    srcidx[c, p, koff[j]:koff[j] + d] = src_sorted[s:e]
                    mask[c, p, koff[j]:koff[j] + d] = 1.0

    h0p = np.zeros((NP, D), np.float32)
    h0p[pos[:N]] = h0
    embT = np.ascontiguousarray(h0p.T).astype(ml_dtypes.bfloat16)

    def wext(W, a_s):
        return np.concatenate([W, (W @ a_s)[:, None]], axis=1).astype(ml_dtypes.bfloat16)

    common = {
        "embT": embT,
        "Wext1": wext(W1, as1), "wad1": (W1 @ ad1)[:, None].astype(ml_dtypes.bfloat16),
        "Wext2": wext(W2, as2), "wad2": (W2 @ ad2)[:, None].astype(ml_dtypes.bfloat16),
        "Wext3": wext(W3, as3), "wad3": (W3 @ ad3)[:, None].astype(ml_dtypes.bfloat16),
        "brep1": np.tile(b1[None, :], (128, 1)).astype(np.float32),
        "brep2": np.tile(b2[None, :], (128, 1)).astype(np.float32),
        "brep3": np.tile(b3[None, :], (128, 1)).astype(np.float32),
        "pcol1": np.full((128, 1), np.float32(p1[0])),
        "pcol2": np.full((128, 1), np.float32(p2[0])),
        "pcol3": np.full((128, 1), np.float32(p3[0])),
        "Wo": np.asarray(Wo).astype(ml_dtypes.bfloat16),
        "borep": np.tile(bo[None, :], (128, 1)).astype(np.float32),
        "ident": np.eye(128, dtype=np.float32),
    }
    in_maps = []
    for c in range(NCORES):
        m = dict(common)
        m["h0T_sh"] = np.ascontiguousarray(
            h0p[c * SHARD:(c + 1) * SHARD].T).astype(ml_dtypes.bfloat16)
        m["srcidx"] = srcidx[c]
        m["maskin"] = mask[c]
        in_maps.append(m)
    return KBAR, in_maps, pos


def kernel(**inputs):
    key = "gat"
    x = inputs["x"]
    edge_index = inputs["edge_index"]
    emb = inputs["emb"]
    weights = tuple(np.asarray(inputs[k], np.float32) for k in (
        "W1", "as1", "ad1", "b1", "p1", "W2", "as2", "ad2", "b2", "p2",
        "W3", "as3", "ad3", "b3", "p3", "Wo", "bo"))
    KBAR, in_maps, pos = _prepare(x, edge_index, emb, weights)

    ck = (key, hash(np.asarray(edge_index).tobytes()))
    if ck not in _cache:
        nc = _build_nc(KBAR)
        _cache[ck] = _Runner(nc, in_maps, NCORES)
    runner = _cache[ck]
    res = runner.results()
    full = np.concatenate([res[c]["out_sh"] for c in range(NCORES)], axis=0)  # [NP, 128]
    return full[pos[:N]].astype(np.float32)


if __name__ == "__main__":
    # quick self-test against the reference (reference runs on CPU backend)
    sys.path.insert(0, '/root/problem')
    import jax
    cpu = jax.devices("cpu")[0]
    with jax.default_device(cpu):
        import reference
        inputs = {k: np.asarray(v) for k, v in reference.setup_inputs().items()}
        exp = np.asarray(reference.reference(**{k: jax.device_put(v, cpu) for k, v in inputs.items()}))
    got = kernel(**inputs)
    err = np.abs(got - exp).max() / (np.abs(exp).max() + 1e-9)
    print("rel err:", err)



# revision 12
# speedup vs baseline: 1.2634x; 1.2634x over previous
"""3-layer GAT + linear head on 8 Trainium2 NeuronCores.

Strategy: destination-sharded edge parallelism, gather-raw-h formulation.
 - Host relabels nodes by in-degree (desc) and deals 128-node chunks
   round-robin to the 8 cores (balanced per-chunk max degrees). Each core
   owns 6272 destination slots (49 chunks of 128).
 - Per layer the gather table holds RAW rows [h_l | es_l] (129 bf16 cols).
   By linearity, out = (sum_k alpha_k h[src_k]) @ W, so W is applied AFTER
   aggregation - one matmul per own chunk, no replicated transform.
 - Layer 0 gathers straight from a host-staged h0 table (no collective);
   only the tiny es0 column (100 KB) is all-gathered. Layers 1,2 exchange
   [h|es] shards with one AllGather each.
 - Aggregation accumulates A^T = sum_k (alpha_k * h_src)^T in PSUM via
   matmul(lhsT=Gs_k, rhs=I), keeping everything feature-major so the W
   transform, per-feature bias (Act bias column) and next-layer es/ed
   matvecs need no transposes; rows for the next table are produced with
   one PE transpose per chunk.
 - One batched indirect DMA per chunk (all K in-edge slots at once).
 - Softmax: ed-add folded into the Prelu bias, exp on Act (Prelu+Exp+Copy
   share one activation table - no table reloads), mask+denominator fused
   in one tensor_tensor_reduce, alpha normalization pre-folded into the
   edge weights.
"""
import sys
sys.path.insert(0, '/opt/trn_rl_repo')
import numpy as np
import ml_dtypes

N = 50000
E = 800000
D = 128
NCORES = 8
NP = 50176           # N padded to 392 chunks of 128
SHARD = NP // NCORES   # 6272
NCHUNK = SHARD // 128  # 49
NTILES = NP // 128     # 392
NEG_SLOPE = 0.2

_cache = {}


def _legalize_single_wait(nc, mybir):
    ctr = 0
    for fn in nc.m.functions:
        for bb in fn.blocks:
            insts = bb.instructions
            out = []
            changed = False
            for inst in insts:
                si = getattr(inst, 'sync_info', None) if hasattr(inst, 'sync_info') else None
                waits = list(si.on_wait) if si and si.on_wait else []
                if len(waits) > 1:
                    eng = inst.engine
                    for w in waits[:-1]:
                        ctr += 1
                        nop = mybir.InstNoOp(name=f"legwait-{ctr}", ins=[], outs=[])
                        nop.engine = eng
                        nop.sync_info = mybir.SyncInfo(on_wait=[w], on_update=[])
                        out.append(nop)
                    inst.sync_info = mybir.SyncInfo(
                        on_wait=waits[-1:], on_update=list(si.on_update or []))
                    changed = True
                out.append(inst)
            if changed:
                bb.instructions = out


def _build_nc(KBAR, alphas):
    """alphas = (p1, p2, p3) PReLU slopes baked as immediates."""
    import concourse.bass as bass
    import concourse.mybir as mybir
    from concourse.tile import TileContext

    SK = int(sum(KBAR))
    f32 = mybir.dt.float32
    bf16 = mybir.dt.bfloat16
    PRELU = mybir.ActivationFunctionType.Prelu
    EXP = mybir.ActivationFunctionType.Exp
    COPY = mybir.ActivationFunctionType.Copy

    nc = bass.Bass()
    # ---- inputs (replicated unless noted)
    h0es = nc.dram_tensor("h0es", [NP, 129], bf16, kind="ExternalInput")
    h0T_sh = nc.dram_tensor("h0T_sh", [128, SHARD], bf16, kind="ExternalInput")  # per-core
    srcidx = nc.dram_tensor("srcidx", [128, SK], mybir.dt.int32, kind="ExternalInput")  # per-core
    maskin = nc.dram_tensor("maskin", [128, SK], bf16, kind="ExternalInput")  # per-core
    W_in, wasd_in, bcol_in = [], [], []
    for l in (1, 2, 3):
        W_in.append(nc.dram_tensor(f"W{l}", [128, 128], bf16, kind="ExternalInput"))
        wasd_in.append(nc.dram_tensor(f"wasd{l}", [128, 2], bf16, kind="ExternalInput"))
        bcol_in.append(nc.dram_tensor(f"bcol{l}", [128, 1], f32, kind="ExternalInput"))
    Wo = nc.dram_tensor("Wo", [128, 128], bf16, kind="ExternalInput")
    bo_rep = nc.dram_tensor("borep", [128, 128], bf16, kind="ExternalInput")
    ones0 = nc.dram_tensor("ones0", [128, 128], bf16, kind="ExternalInput")
    ident = nc.dram_tensor("ident", [128, 128], bf16, kind="ExternalInput")
    out_sh = nc.dram_tensor("out_sh", [SHARD, 128], f32, kind="ExternalOutput")
    # ---- internals
    es0_st = nc.dram_tensor("es0_st", [SHARD, 1], bf16, kind="Internal")
    es0_full = nc.dram_tensor("es0_full", [NP, 1], bf16, kind="Internal", addr_space="Shared")
    Tst = nc.dram_tensor("Tst", [SHARD, 129], bf16, kind="Internal")
    Table = nc.dram_tensor("Table", [NP, 129], bf16, kind="Internal", addr_space="Shared")

    koff = np.zeros(NCHUNK + 1, np.int64)
    for j in range(NCHUNK):
        koff[j + 1] = koff[j] + KBAR[j]

    with TileContext(nc) as tc:
        with (
            tc.tile_pool(name="consts", bufs=1) as cpool,
            tc.tile_pool(name="gbuf", bufs=4) as gpool,
            tc.tile_pool(name="gsb", bufs=3) as gspool,
            tc.tile_pool(name="sca", bufs=4) as spool,
            tc.tile_pool(name="ev", bufs=3) as epool,
            tc.tile_pool(name="psa", bufs=2, space="PSUM") as psa,
            tc.tile_pool(name="psw", bufs=2, space="PSUM") as psw,
            tc.tile_pool(name="pst", bufs=2, space="PSUM") as pst,
            tc.tile_pool(name="pse", bufs=1, space="PSUM") as pse,
        ):
            # ---- load constants
            ident_sb = cpool.tile([128, 128], bf16)
            nc.sync.dma_start(ident_sb[:], ident[:])
            ones0_sb = cpool.tile([128, 128], bf16)
            nc.sync.dma_start(ones0_sb[:], ones0[:])
            Wo_sb = cpool.tile([128, 128], bf16)
            nc.sync.dma_start(Wo_sb[:], Wo[:])
            borep_sb = cpool.tile([128, 128], bf16)
            nc.sync.dma_start(borep_sb[:], bo_rep[:])
            W_sb, wasd_sb, bcol_sb = [], [], []
            for l in range(3):
                t = cpool.tile([128, 128], bf16, tag=f"W{l}")
                nc.sync.dma_start(t[:], W_in[l][:])
                W_sb.append(t)
                t = cpool.tile([128, 2], bf16, tag=f"wasd{l}")
                nc.sync.dma_start(t[:], wasd_in[l][:])
                wasd_sb.append(t)
                t = cpool.tile([128, 1], f32, tag=f"bcol{l}")
                nc.sync.dma_start(t[:], bcol_in[l][:])
                bcol_sb.append(t)
            srcidx_sb = cpool.tile([128, SK], mybir.dt.int32)
            nc.sync.dma_start(srcidx_sb[:], srcidx[:])
            mask_sb = cpool.tile([128, SK], bf16)
            nc.sync.dma_start(mask_sb[:], maskin[:])
            h0T_all = cpool.tile([128, SHARD], bf16)
            nc.sync.dma_start(h0T_all[:], h0T_sh[:])

            # per-layer own-shard ed columns ([128, NCHUNK] f32)
            ed_sb = [cpool.tile([128, NCHUNK], f32, tag=f"ed{l}", name=f"ed{l}")
                     for l in range(3)]

            # ---- prologue: es0/ed0 for own shard, tiny AllGather of es0
            pE = pse.tile([128, 2 * NCHUNK], f32, tag="pE0")
            for j in range(NCHUNK):
                nc.tensor.matmul(pE[:, 2 * j:2 * j + 2],
                                 lhsT=h0T_all[:, j * 128:(j + 1) * 128],
                                 rhs=wasd_sb[0][:], start=True, stop=True)
            esed0 = cpool.tile([128, 2 * NCHUNK], f32, tag="esed0")
            nc.scalar.activation(esed0[:], pE[:], COPY)
            e3 = esed0[:].rearrange("p (j e) -> p j e", e=2)
            # ed0 column for own chunks
            nc.vector.tensor_scalar(out=ed_sb[0][:], in0=e3[:, :, 1],
                                    scalar1=0.0, scalar2=None,
                                    op0=mybir.AluOpType.add)
            es0col = cpool.tile([128, NCHUNK], bf16, tag="es0col")
            nc.vector.tensor_scalar(out=es0col[:], in0=e3[:, :, 0],
                                    scalar1=0.0, scalar2=None,
                                    op0=mybir.AluOpType.add)
            nc.sync.dma_start(
                es0_st[:].rearrange("(j p) e -> p j e", p=128),
                es0col[:].rearrange("p (j e) -> p j e", e=1))
            nc.gpsimd.collective_compute(
                "AllGather", mybir.AluOpType.bypass,
                ins=[es0_st[:]], outs=[es0_full[:]],
                replica_groups=[list(range(NCORES))],
            )
            # layer-0 table: [h0 | es0] assembled in DRAM
            nc.sync.dma_start(Table[:], h0es[:])
            with nc.allow_non_contiguous_dma(reason="es0 column scatter into table"):
                nc.sync.dma_start(Table[:, 128:129], es0_full[:])

            # ---- layers
            for layer in range(3):
                for j in range(NCHUNK):
                    K = int(KBAR[j])
                    o0 = int(koff[j])
                    Gc = gpool.tile([128, K * 129], bf16, tag="Gc")
                    G3 = Gc[:].rearrange("p (k e) -> p k e", e=129)
                    for k in range(K):
                        nc.gpsimd.indirect_dma_start(
                            out=G3[:, k, :], out_offset=None, in_=Table[:],
                            in_offset=bass.IndirectOffsetOnAxis(
                                ap=srcidx_sb[:, o0 + k:o0 + k + 1], axis=0))
                    es_view = G3[:, :, 128]
                    GH = G3[:, :, 0:128]

                    # softmax over the K in-edge slots (exact, masked)
                    tL = spool.tile([128, K], f32, tag="tL")
                    nc.scalar.activation(tL[:], es_view, PRELU,
                                         bias=ed_sb[layer][:, j:j + 1], scale=1.0,
                                         alpha=NEG_SLOPE)
                    mx = spool.tile([128, 1], f32, tag="mx")
                    nc.vector.tensor_reduce(mx[:], tL[:], axis=mybir.AxisListType.X,
                                            op=mybir.AluOpType.max)
                    ngm = spool.tile([128, 1], f32, tag="ngm")
                    nc.vector.tensor_scalar(out=ngm[:], in0=mx[:], scalar1=-1.0,
                                            scalar2=None, op0=mybir.AluOpType.mult)
                    wE = spool.tile([128, K], bf16, tag="wE")
                    nc.scalar.activation(wE[:], tL[:], EXP, bias=ngm[:, 0:1], scale=1.0)
                    w2 = spool.tile([128, K], bf16, tag="w2")
                    nc.vector.tensor_tensor(out=w2[:], in0=wE[:],
                                            in1=mask_sb[:, o0:o0 + K],
                                            op=mybir.AluOpType.mult)
                    zz = spool.tile([128, 1], f32, tag="zz")
                    nc.vector.tensor_reduce(zz[:], w2[:], axis=mybir.AxisListType.X,
                                            op=mybir.AluOpType.add)
                    zc = spool.tile([128, 1], f32, tag="zc")
                    nc.vector.tensor_scalar(out=zc[:], in0=zz[:], scalar1=1e-30,
                                            scalar2=None, op0=mybir.AluOpType.max)
                    zi = spool.tile([128, 1], f32, tag="zi")
                    nc.vector.reciprocal(zi[:], zc[:])
                    al = spool.tile([128, K], bf16, tag="al")
                    nc.vector.tensor_scalar(out=al[:], in0=w2[:], scalar1=zi[:, 0:1],
                                            scalar2=None, op0=mybir.AluOpType.mult)
                    Gs = gspool.tile([128, K * 128], bf16, tag="Gs")
                    Gs3 = Gs[:].rearrange("p (k e) -> p k e", e=128)
                    nc.vector.tensor_tensor(
                        out=Gs3[:, :, :], in0=GH,
                        in1=al[:].unsqueeze(2).broadcast_to((128, K, 128)),
                        op=mybir.AluOpType.mult)

                    # A^T = sum_k (alpha_k h_src)^T accumulated in PSUM
                    pa = psa.tile([128, 128], f32, tag="pa")
                    for k in range(K):
                        nc.tensor.matmul(pa[:], lhsT=Gs3[:, k, :], rhs=ident_sb[:],
                                         start=(k == 0), stop=(k == K - 1))
                    At = epool.tile([128, 128], bf16, tag="At")
                    nc.scalar.activation(At[:], pa[:], COPY)
                    # h_next^T = prelu(W^T A^T + b)
                    ph = psw.tile([128, 128], f32, tag="ph")
                    nc.tensor.matmul(ph[:], lhsT=W_sb[layer][:], rhs=At[:],
                                     start=True, stop=True)
                    hT = epool.tile([128, 128], bf16, tag="hT")
                    nc.scalar.activation(hT[:], ph[:], PRELU,
                                         bias=bcol_sb[layer][:, 0:1], scale=1.0,
                                         alpha=float(alphas[layer]))

                    if layer < 2:
                        # es/ed for next layer + row-major [h|es] table rows
                        pe2 = pse.tile([128, 2], f32, tag="pe2")
                        nc.tensor.matmul(pe2[:], lhsT=hT[:], rhs=wasd_sb[layer + 1][:],
                                         start=True, stop=True)
                        pt = pst.tile([128, 128], bf16, tag="pt")
                        nc.tensor.transpose(pt[:], hT[:], ident_sb[:])
                        stage = epool.tile([128, 129], bf16, tag="stage")
                        nc.scalar.activation(stage[:, 0:128], pt[:], COPY)
                        nc.scalar.activation(stage[:, 128:129], pe2[:, 0:1], COPY)
                        nc.vector.tensor_scalar(out=ed_sb[layer + 1][:, j:j + 1],
                                                in0=pe2[:, 1:2], scalar1=0.0,
                                                scalar2=None, op0=mybir.AluOpType.add)
                        nc.sync.dma_start(Tst[j * 128:(j + 1) * 128, :], stage[:])
                    else:
                        # final linear fused into the layer-2 epilogue
                        po = psw.tile([128, 128], f32, tag="ph")
                        nc.tensor.matmul(po[:], lhsT=hT[:], rhs=Wo_sb[:],
                                         start=True, stop=False)
                        nc.tensor.matmul(po[:], lhsT=ones0_sb[:], rhs=borep_sb[:],
                                         start=False, stop=True)
                        oo = epool.tile([128, 128], f32, tag="oo")
                        nc.scalar.activation(oo[:], po[:], COPY)
                        nc.sync.dma_start(out_sh[j * 128:(j + 1) * 128, :], oo[:])

                if layer < 2:
                    nc.gpsimd.collective_compute(
                        "AllGather", mybir.AluOpType.bypass,
                        ins=[Tst[:]], outs=[Table[:]],
                        replica_groups=[list(range(NCORES))],
                    )

    _legalize_single_wait(nc, mybir)
    return nc


class _Runner:
    def __init__(self, nc, in_maps, n_cores):
        import jax
        import concourse.mybir as mybir
        from concourse.bass2jax import (_bass_exec_p, partition_id_tensor,
                                        install_neuronx_cc_hook)
        from jax.sharding import Mesh, PartitionSpec
        from jax.experimental.shard_map import shard_map
        install_neuronx_cc_hook()
        self.jax = jax
        self.n_cores = n_cores
        in_names, out_names, out_avals, zero_outs = [], [], [], []
        partition_name = nc.partition_id_tensor.name if nc.partition_id_tensor else None
        for alloc in nc.m.functions[0].allocations:
            if not isinstance(alloc, mybir.MemoryLocationSet):
                continue
            name = alloc.memorylocations[0].name
            if alloc.kind == "ExternalInput":
                if name != partition_name:
                    in_names.append(name)
            elif alloc.kind == "ExternalOutput":
                shape = tuple(alloc.tensor_shape)
                dtype = mybir.dt.np(alloc.dtype)
                out_names.append(name)
                out_avals.append(jax.core.ShapedArray(shape, dtype))
                zero_outs.append(np.zeros(shape, dtype))
        n_params = len(in_names)
        self.out_names, self.out_avals = out_names, out_avals
        all_in = list(in_names) + list(out_names)
        if partition_name is not None:
            all_in.append(partition_name)

        def _body(*args):
            operands = list(args)
            if partition_name is not None:
                operands.append(partition_id_tensor())
            outs = _bass_exec_p.bind(
                *operands, out_avals=tuple(out_avals), in_names=tuple(all_in),
                out_names=tuple(out_names), lowering_input_output_aliases=(),
                sim_require_finite=False, sim_require_nnan=False, nc=nc)
            return tuple(outs)

        devices = jax.devices()[:n_cores]
        mesh = Mesh(np.asarray(devices), ("core",))
        self.fn = jax.jit(
            shard_map(_body, mesh=mesh,
                      in_specs=(PartitionSpec("core"),) * (n_params + len(out_names)),
                      out_specs=(PartitionSpec("core"),) * len(out_names),
                      check_rep=False),
            keep_unused=True)
        per_core = [[np.asarray(m[nm]) for nm in in_names] for m in in_maps]
        concat_in = [np.concatenate([per_core[c][i] for c in range(n_cores)], axis=0)
                     for i in range(n_params)]
        concat_zeros = [np.zeros((n_cores * z.shape[0], *z.shape[1:]), z.dtype)
                        for z in zero_outs]
        sh = jax.sharding.NamedSharding(mesh, PartitionSpec("core"))
        self.dev_args = [jax.device_put(a, sh) for a in concat_in + concat_zeros]

    def run_raw(self):
        return self.fn(*self.dev_args)

    def results(self):
        outs = self.run_raw()
        self.jax.block_until_ready(outs)
        return [
            {nm: np.asarray(outs[i]).reshape(self.n_cores, *self.out_avals[i].shape)[c]
             for i, nm in enumerate(self.out_names)}
            for c in range(self.n_cores)]


def _prepare(x, edge_index, emb, weights):
    """Host-side: relabel, chunk, schedule, build per-core inputs."""
    (W1, as1, ad1, b1, p1, W2, as2, ad2, b2, p2,
     W3, as3, ad3, b3, p3, Wo, bo) = weights
    h0 = np.asarray(emb)[np.asarray(x)]  # [N, D] f32
    src = np.asarray(edge_index[0], np.int64)
    dst = np.asarray(edge_index[1], np.int64)
    src = np.concatenate([src, np.arange(N, dtype=np.int64)])
    dst = np.concatenate([dst, np.arange(N, dtype=np.int64)])

    deg = np.bincount(dst, minlength=NP)  # pad nodes deg 0
    order = np.argsort(-deg, kind="stable")  # [NP]
    pos = np.empty(NP, np.int64)
    # chunk rank r -> core r%8, local j=r//8; pos = core*SHARD + j*128 + i
    for r in range(NTILES):
        nodes = order[r * 128:(r + 1) * 128]
        core, j = r % NCORES, r // NCORES
        pos[nodes] = core * SHARD + j * 128 + np.arange(128)

    srcp = pos[src]
    dstp = pos[dst]

    # group edges by dst position
    o = np.argsort(dstp, kind="stable")
    dst_sorted = dstp[o]
    src_sorted = srcp[o]
    starts = np.searchsorted(dst_sorted, np.arange(NP))
    ends = np.searchsorted(dst_sorted, np.arange(NP) + 1)
    degs_pos = ends - starts  # degree by position

    # KBAR[j] = max degree among all cores' chunks with local index j
    dp = degs_pos.reshape(NCORES, NCHUNK, 128)
    KBAR = dp.max(axis=(0, 2)).astype(np.int64)  # [NCHUNK]
    KBAR = np.maximum(KBAR, 1)
    SK = int(KBAR.sum())

    srcidx = np.zeros((NCORES, 128, SK), np.int32)
    mask = np.zeros((NCORES, 128, SK), np.float32)
    koff = np.concatenate([[0], np.cumsum(KBAR)])
    for c in range(NCORES):
        for j in range(NCHUNK):
            base = c * SHARD + j * 128
            K = int(KBAR[j])
            for p in range(128):
                s, e = starts[base + p], ends[base + p]
                d = e - s
                if d:
                    srcidx[c, p, koff[j]:koff[j] + d] = src_sorted[s:e]
                    mask[c, p, koff[j]:koff[j] + d] = 1.0

    h0p = np.zeros((NP, D), np.float32)
    h0p[pos[:N]] = h0
    bf = ml_dtypes.bfloat16
    h0es = np.zeros((NP, 129), np.float32)
    h0es[:, 0:128] = h0p
    h0es = h0es.astype(bf)

    def wasd(W, a_s, a_d):
        return np.stack([W @ a_s, W @ a_d], axis=1).astype(bf)

    ones0 = np.zeros((128, 128), np.float32)
    ones0[0, :] = 1.0
    common = {
        "h0es": h0es,
        "W1": W1.astype(bf), "wasd1": wasd(W1, as1, ad1),
        "W2": W2.astype(bf), "wasd2": wasd(W2, as2, ad2),
        "W3": W3.astype(bf), "wasd3": wasd(W3, as3, ad3),
        "bcol1": b1[:, None].astype(np.float32),
        "bcol2": b2[:, None].astype(np.float32),
        "bcol3": b3[:, None].astype(np.float32),
        "Wo": np.asarray(Wo).astype(bf),
        "borep": np.tile(bo[None, :], (128, 1)).astype(bf),
        "ones0": ones0.astype(bf),
        "ident": np.eye(128, dtype=np.float32).astype(bf),
    }
    in_maps = []
    for c in range(NCORES):
        m = dict(common)
        m["h0T_sh"] = np.ascontiguousarray(
            h0p[c * SHARD:(c + 1) * SHARD].T).astype(bf)
        m["srcidx"] = srcidx[c]
        m["maskin"] = mask[c].astype(bf)
        in_maps.append(m)
    alphas = (float(np.asarray(p1).ravel()[0]),
              float(np.asarray(p2).ravel()[0]),
              float(np.asarray(p3).ravel()[0]))
    return KBAR, in_maps, pos, alphas


def kernel(**inputs):
    x = inputs["x"]
    edge_index = inputs["edge_index"]
    emb = inputs["emb"]
    weights = tuple(np.asarray(inputs[k], np.float32) for k in (
        "W1", "as1", "ad1", "b1", "p1", "W2", "as2", "ad2", "b2", "p2",
        "W3", "as3", "ad3", "b3", "p3", "Wo", "bo"))
    KBAR, in_maps, pos, alphas = _prepare(x, edge_index, emb, weights)

    import hashlib
    hsh = hashlib.sha256()
    hsh.update(np.asarray(edge_index).tobytes())
    hsh.update(np.asarray(emb, np.float32).tobytes())
    hsh.update(np.asarray(x).tobytes())
    for w in weights:
        hsh.update(w.tobytes())
    ck = ("gat2", hsh.hexdigest())
    if ck not in _cache:
        nc = _build_nc(KBAR, alphas)
        _cache[ck] = _Runner(nc, in_maps, NCORES)
    runner = _cache[ck]
    res = runner.results()
    full = np.concatenate([res[c]["out_sh"] for c in range(NCORES)], axis=0)  # [NP, 128]
    return full[pos[:N]].astype(np.float32)


if __name__ == "__main__":
    # quick self-test against the reference (reference runs on CPU backend)
    sys.path.insert(0, '/root/problem')
    import jax
    cpu = jax.devices("cpu")[0]
    with jax.default_device(cpu):
        import reference
        inputs = {k: np.asarray(v) for k, v in reference.setup_inputs().items()}
        exp = np.asarray(reference.reference(**{k: jax.device_put(v, cpu) for k, v in inputs.items()}))
    got = kernel(**inputs)
    err = np.abs(got - exp).max() / (np.abs(exp).max() + 1e-9)
    print("rel err:", err)


# revision 21
# speedup vs baseline: 1.4657x; 1.1601x over previous
"""3-layer GAT + linear head on 8 Trainium2 NeuronCores.

Strategy: destination-sharded edge parallelism, gather-raw-h formulation.
 - Host relabels nodes by in-degree (desc) and deals 128-node chunks
   round-robin to the 8 cores (balanced per-chunk max degrees). Each core
   owns 6272 destination slots (49 chunks of 128).
 - Per layer the gather table holds RAW rows [h_l | es_l] (129 bf16 cols).
   By linearity, out = (sum_k alpha_k h[src_k]) @ W, so W is applied AFTER
   aggregation - one matmul per own chunk, no replicated transform.
 - Layer 0 gathers straight from a host-staged h0 table (no collective);
   only the tiny es0 column (100 KB) is all-gathered. Layers 1,2 exchange
   [h|es] shards with one AllGather each.
 - Aggregation accumulates A^T = sum_k (alpha_k * h_src)^T in PSUM via
   matmul(lhsT=Gs_k, rhs=I), keeping everything feature-major so the W
   transform, per-feature bias (Act bias column) and next-layer es/ed
   matvecs need no transposes; rows for the next table are produced with
   one PE transpose per chunk.
 - One batched indirect DMA per chunk (all K in-edge slots at once).
 - Softmax: ed-add folded into the Prelu bias, exp on Act (Prelu+Exp+Copy
   share one activation table - no table reloads), mask+denominator fused
   in one tensor_tensor_reduce, alpha normalization pre-folded into the
   edge weights.
"""
import sys
sys.path.insert(0, '/opt/trn_rl_repo')
import numpy as np
import ml_dtypes

N = 50000
E = 800000
D = 128
NCORES = 8
NP = 50176           # N padded to 392 chunks of 128
SHARD = NP // NCORES   # 6272
NCHUNK = SHARD // 128  # 49
NTILES = NP // 128     # 392
NEG_SLOPE = 0.2

_cache = {}


def _legalize_single_wait(nc, mybir):
    ctr = 0
    for fn in nc.m.functions:
        for bb in fn.blocks:
            insts = bb.instructions
            out = []
            changed = False
            for inst in insts:
                si = getattr(inst, 'sync_info', None) if hasattr(inst, 'sync_info') else None
                waits = list(si.on_wait) if si and si.on_wait else []
                if len(waits) > 1:
                    eng = inst.engine
                    for w in waits[:-1]:
                        ctr += 1
                        nop = mybir.InstNoOp(name=f"legwait-{ctr}", ins=[], outs=[])
                        nop.engine = eng
                        nop.sync_info = mybir.SyncInfo(on_wait=[w], on_update=[])
                        out.append(nop)
                    inst.sync_info = mybir.SyncInfo(
                        on_wait=waits[-1:], on_update=list(si.on_update or []))
                    changed = True
                out.append(inst)
            if changed:
                bb.instructions = out


def _build_nc(KBAR, alphas):
    """alphas = (p1, p2, p3) PReLU slopes baked as immediates."""
    import concourse.bass as bass
    import concourse.mybir as mybir
    from concourse.tile import TileContext

    SK = int(sum(KBAR))
    f32 = mybir.dt.float32
    bf16 = mybir.dt.bfloat16
    PRELU = mybir.ActivationFunctionType.Prelu
    EXP = mybir.ActivationFunctionType.Exp
    COPY = mybir.ActivationFunctionType.Copy

    nc = bass.Bass()
    # ---- inputs (replicated unless noted)
    # layer-0 gather pre-done on host: edge-major h0 rows, plus transposed
    # windows for the per-slot es0 matvecs (both per-core)
    G0h = nc.dram_tensor("G0h", [128, SK * 128], bf16, kind="ExternalInput")
    G0t = nc.dram_tensor("G0t", [128, SK * 128], bf16, kind="ExternalInput")
    h0T_sh = nc.dram_tensor("h0T_sh", [128, SHARD], bf16, kind="ExternalInput")  # per-core
    srcidx = nc.dram_tensor("srcidx", [128, SK], mybir.dt.int32, kind="ExternalInput")  # per-core
    maskin = nc.dram_tensor("maskin", [128, SK], bf16, kind="ExternalInput")  # per-core
    W_in, wasd_in, bcol_in = [], [], []
    for l in (1, 2, 3):
        W_in.append(nc.dram_tensor(f"W{l}", [128, 128], bf16, kind="ExternalInput"))
        wasd_in.append(nc.dram_tensor(f"wasd{l}", [128, 2], bf16, kind="ExternalInput"))
        bcol_in.append(nc.dram_tensor(f"bcol{l}", [128, 1], f32, kind="ExternalInput"))
    Wo = nc.dram_tensor("Wo", [128, 128], bf16, kind="ExternalInput")
    bo_rep = nc.dram_tensor("borep", [128, 128], bf16, kind="ExternalInput")
    ones0 = nc.dram_tensor("ones0", [128, 128], bf16, kind="ExternalInput")
    ident = nc.dram_tensor("ident", [128, 128], bf16, kind="ExternalInput")
    out_sh = nc.dram_tensor("out_sh", [SHARD, 128], f32, kind="ExternalOutput")
    # ---- internals
    Tst = nc.dram_tensor("Tst", [SHARD, 129], bf16, kind="Internal")
    Table = nc.dram_tensor("Table", [NP, 129], bf16, kind="Internal", addr_space="Shared")

    koff = np.zeros(NCHUNK + 1, np.int64)
    for j in range(NCHUNK):
        koff[j + 1] = koff[j] + KBAR[j]

    with TileContext(nc) as tc:
        with (
            tc.tile_pool(name="consts", bufs=1) as cpool,
            tc.tile_pool(name="gbuf", bufs=4) as gpool,
            tc.tile_pool(name="gtb", bufs=3) as gtpool,
            tc.tile_pool(name="gsb", bufs=3) as gspool,
            tc.tile_pool(name="sca", bufs=4) as spool,
            tc.tile_pool(name="ev", bufs=3) as epool,
            tc.tile_pool(name="psa", bufs=2, space="PSUM") as psa,
            tc.tile_pool(name="psw", bufs=2, space="PSUM") as psw,
            tc.tile_pool(name="pst", bufs=1, space="PSUM") as pst,
            tc.tile_pool(name="pse", bufs=2, space="PSUM") as pse,
        ):
            # ---- load constants
            ident_sb = cpool.tile([128, 128], bf16)
            nc.sync.dma_start(ident_sb[:], ident[:])
            ones0_sb = cpool.tile([128, 128], bf16)
            nc.sync.dma_start(ones0_sb[:], ones0[:])
            Wo_sb = cpool.tile([128, 128], bf16)
            nc.sync.dma_start(Wo_sb[:], Wo[:])
            borep_sb = cpool.tile([128, 128], bf16)
            nc.sync.dma_start(borep_sb[:], bo_rep[:])
            W_sb, wasd_sb, bcol_sb = [], [], []
            for l in range(3):
                t = cpool.tile([128, 128], bf16, tag=f"W{l}")
                nc.sync.dma_start(t[:], W_in[l][:])
                W_sb.append(t)
                t = cpool.tile([128, 2], bf16, tag=f"wasd{l}")
                nc.sync.dma_start(t[:], wasd_in[l][:])
                wasd_sb.append(t)
                t = cpool.tile([128, 1], f32, tag=f"bcol{l}")
                nc.sync.dma_start(t[:], bcol_in[l][:])
                bcol_sb.append(t)
            srcidx_sb = cpool.tile([128, SK], mybir.dt.int32)
            nc.sync.dma_start(srcidx_sb[:], srcidx[:])
            mask_sb = cpool.tile([128, SK], bf16)
            nc.sync.dma_start(mask_sb[:], maskin[:])
            h0T_all = cpool.tile([128, SHARD], bf16)
            nc.sync.dma_start(h0T_all[:], h0T_sh[:])

            # per-layer own-shard ed columns ([128, NCHUNK] f32)
            ed_sb = [cpool.tile([128, NCHUNK], f32, tag=f"ed{l}", name=f"ed{l}")
                     for l in range(3)]

            # ---- prologue: ed0 for own shard (ed0[d] = h0[d] @ (W1 ad1))
            pE = pse.tile([128, NCHUNK], f32, tag="pes")
            for j in range(NCHUNK):
                nc.tensor.matmul(pE[:, j:j + 1],
                                 lhsT=h0T_all[:, j * 128:(j + 1) * 128],
                                 rhs=wasd_sb[0][:, 1:2], start=True, stop=True)
            nc.scalar.activation(ed_sb[0][:], pE[:], COPY)

            # ---- layers
            for layer in range(3):
                for j in range(NCHUNK):
                    K = int(KBAR[j])
                    o0 = int(koff[j])
                    if layer == 0:
                        # host-pregathered edge-major h0 (+ transposed windows)
                        Gc = gpool.tile([128, K * 128], bf16, tag="Gc")
                        nc.sync.dma_start(
                            Gc[:], G0h[:, o0 * 128:(o0 + K) * 128])
                        GH = Gc[:].rearrange("p (k e) -> p k e", e=128)
                        Gt = gtpool.tile([128, K * 128], bf16, tag="Gt")
                        nc.sync.dma_start(
                            Gt[:], G0t[:, o0 * 128:(o0 + K) * 128])
                        pes = pse.tile([128, K], f32, tag="pes")
                        for k in range(K):
                            nc.tensor.matmul(
                                pes[:, k:k + 1],
                                lhsT=Gt[:, k * 128:(k + 1) * 128],
                                rhs=wasd_sb[0][:, 0:1], start=True, stop=True)
                        es_view = pes[:]
                    else:
                        Gc = gpool.tile([128, K * 129], bf16, tag="Gc")
                        G3 = Gc[:].rearrange("p (k e) -> p k e", e=129)
                        for k in range(K):
                            nc.gpsimd.indirect_dma_start(
                                out=G3[:, k, :], out_offset=None, in_=Table[:],
                                in_offset=bass.IndirectOffsetOnAxis(
                                    ap=srcidx_sb[:, o0 + k:o0 + k + 1], axis=0))
                        es_view = G3[:, :, 128]
                        GH = G3[:, :, 0:128]

                    # softmax over the K in-edge slots (exact, masked)
                    tL = spool.tile([128, K], f32, tag="tL")
                    nc.scalar.activation(tL[:], es_view, PRELU,
                                         bias=ed_sb[layer][:, j:j + 1], scale=1.0,
                                         alpha=NEG_SLOPE)
                    mx = spool.tile([128, 1], f32, tag="mx")
                    nc.vector.tensor_reduce(mx[:], tL[:], axis=mybir.AxisListType.X,
                                            op=mybir.AluOpType.max)
                    ngm = spool.tile([128, 1], f32, tag="ngm")
                    nc.vector.tensor_scalar(out=ngm[:], in0=mx[:], scalar1=-1.0,
                                            scalar2=None, op0=mybir.AluOpType.mult)
                    wE = spool.tile([128, K], bf16, tag="wE")
                    nc.scalar.activation(wE[:], tL[:], EXP, bias=ngm[:, 0:1], scale=1.0)
                    w2 = spool.tile([128, K], bf16, tag="w2")
                    nc.vector.tensor_tensor(out=w2[:], in0=wE[:],
                                            in1=mask_sb[:, o0:o0 + K],
                                            op=mybir.AluOpType.mult)
                    zz = spool.tile([128, 1], f32, tag="zz")
                    nc.vector.tensor_reduce(zz[:], w2[:], axis=mybir.AxisListType.X,
                                            op=mybir.AluOpType.add)
                    zc = spool.tile([128, 1], f32, tag="zc")
                    nc.vector.tensor_scalar(out=zc[:], in0=zz[:], scalar1=1e-30,
                                            scalar2=None, op0=mybir.AluOpType.max)
                    zi = spool.tile([128, 1], f32, tag="zi")
                    nc.vector.reciprocal(zi[:], zc[:])
                    al = spool.tile([128, K], bf16, tag="al")
                    nc.vector.tensor_scalar(out=al[:], in0=w2[:], scalar1=zi[:, 0:1],
                                            scalar2=None, op0=mybir.AluOpType.mult)
                    Gs = gspool.tile([128, K * 128], bf16, tag="Gs")
                    Gs3 = Gs[:].rearrange("p (k e) -> p k e", e=128)
                    nc.vector.tensor_tensor(
                        out=Gs3[:, :, :], in0=GH,
                        in1=al[:].unsqueeze(2).broadcast_to((128, K, 128)),
                        op=mybir.AluOpType.mult)

                    # A^T = sum_k (alpha_k h_src)^T accumulated in PSUM
                    pa = psa.tile([128, 128], f32, tag="pa")
                    for k in range(K):
                        nc.tensor.matmul(pa[:], lhsT=Gs3[:, k, :], rhs=ident_sb[:],
                                         start=(k == 0), stop=(k == K - 1))
                    At = epool.tile([128, 128], bf16, tag="At")
                    nc.scalar.activation(At[:], pa[:], COPY)
                    # h_next^T = prelu(W^T A^T + b)
                    ph = psw.tile([128, 128], f32, tag="ph")
                    nc.tensor.matmul(ph[:], lhsT=W_sb[layer][:], rhs=At[:],
                                     start=True, stop=True)
                    hT = epool.tile([128, 128], bf16, tag="hT")
                    nc.scalar.activation(hT[:], ph[:], PRELU,
                                         bias=bcol_sb[layer][:, 0:1], scale=1.0,
                                         alpha=float(alphas[layer]))

                    if layer < 2:
                        # es/ed for next layer + row-major [h|es] table rows
                        pe2 = pse.tile([128, 2], f32, tag="pes")
                        nc.tensor.matmul(pe2[:], lhsT=hT[:], rhs=wasd_sb[layer + 1][:],
                                         start=True, stop=True)
                        pt = pst.tile([128, 128], bf16, tag="pt")
                        nc.tensor.transpose(pt[:], hT[:], ident_sb[:])
                        stage = epool.tile([128, 129], bf16, tag="stage")
                        nc.scalar.activation(stage[:, 0:128], pt[:], COPY)
                        nc.scalar.activation(stage[:, 128:129], pe2[:, 0:1], COPY)
                        nc.vector.tensor_scalar(out=ed_sb[layer + 1][:, j:j + 1],
                                                in0=pe2[:, 1:2], scalar1=0.0,
                                                scalar2=None, op0=mybir.AluOpType.add)
                        nc.sync.dma_start(Tst[j * 128:(j + 1) * 128, :], stage[:])
                    else:
                        # final linear fused into the layer-2 epilogue
                        po = psw.tile([128, 128], f32, tag="ph")
                        nc.tensor.matmul(po[:], lhsT=hT[:], rhs=Wo_sb[:],
                                         start=True, stop=False)
                        nc.tensor.matmul(po[:], lhsT=ones0_sb[:], rhs=borep_sb[:],
                                         start=False, stop=True)
                        oo = epool.tile([128, 128], f32, tag="oo")
                        nc.scalar.activation(oo[:], po[:], COPY)
                        nc.sync.dma_start(out_sh[j * 128:(j + 1) * 128, :], oo[:])

                if layer < 2:
                    nc.gpsimd.collective_compute(
                        "AllGather", mybir.AluOpType.bypass,
                        ins=[Tst[:]], outs=[Table[:]],
                        replica_groups=[list(range(NCORES))],
                    )

    _legalize_single_wait(nc, mybir)
    return nc


class _Runner:
    def __init__(self, nc, in_maps, n_cores):
        import jax
        import concourse.mybir as mybir
        from concourse.bass2jax import (_bass_exec_p, partition_id_tensor,
                                        install_neuronx_cc_hook)
        from jax.sharding import Mesh, PartitionSpec
        from jax.experimental.shard_map import shard_map
        install_neuronx_cc_hook()
        self.jax = jax
        self.n_cores = n_cores
        in_names, out_names, out_avals, zero_outs = [], [], [], []
        partition_name = nc.partition_id_tensor.name if nc.partition_id_tensor else None
        for alloc in nc.m.functions[0].allocations:
            if not isinstance(alloc, mybir.MemoryLocationSet):
                continue
            name = alloc.memorylocations[0].name
            if alloc.kind == "ExternalInput":
                if name != partition_name:
                    in_names.append(name)
            elif alloc.kind == "ExternalOutput":
                shape = tuple(alloc.tensor_shape)
                dtype = mybir.dt.np(alloc.dtype)
                out_names.append(name)
                out_avals.append(jax.core.ShapedArray(shape, dtype))
                zero_outs.append(np.zeros(shape, dtype))
        n_params = len(in_names)
        self.out_names, self.out_avals = out_names, out_avals
        all_in = list(in_names) + list(out_names)
        if partition_name is not None:
            all_in.append(partition_name)

        def _body(*args):
            operands = list(args)
            if partition_name is not None:
                operands.append(partition_id_tensor())
            outs = _bass_exec_p.bind(
                *operands, out_avals=tuple(out_avals), in_names=tuple(all_in),
                out_names=tuple(out_names), lowering_input_output_aliases=(),
                sim_require_finite=False, sim_require_nnan=False, nc=nc)
            return tuple(outs)

        devices = jax.devices()[:n_cores]
        mesh = Mesh(np.asarray(devices), ("core",))
        self.fn = jax.jit(
            shard_map(_body, mesh=mesh,
                      in_specs=(PartitionSpec("core"),) * (n_params + len(out_names)),
                      out_specs=(PartitionSpec("core"),) * len(out_names),
                      check_rep=False),
            keep_unused=True)
        per_core = [[np.asarray(m[nm]) for nm in in_names] for m in in_maps]
        concat_in = [np.concatenate([per_core[c][i] for c in range(n_cores)], axis=0)
                     for i in range(n_params)]
        concat_zeros = [np.zeros((n_cores * z.shape[0], *z.shape[1:]), z.dtype)
                        for z in zero_outs]
        sh = jax.sharding.NamedSharding(mesh, PartitionSpec("core"))
        self.dev_args = [jax.device_put(a, sh) for a in concat_in + concat_zeros]

    def run_raw(self):
        return self.fn(*self.dev_args)

    def results(self):
        outs = self.run_raw()
        self.jax.block_until_ready(outs)
        return [
            {nm: np.asarray(outs[i]).reshape(self.n_cores, *self.out_avals[i].shape)[c]
             for i, nm in enumerate(self.out_names)}
            for c in range(self.n_cores)]


def _prepare(x, edge_index, emb, weights):
    """Host-side: relabel, chunk, schedule, build per-core inputs."""
    (W1, as1, ad1, b1, p1, W2, as2, ad2, b2, p2,
     W3, as3, ad3, b3, p3, Wo, bo) = weights
    h0 = np.asarray(emb)[np.asarray(x)]  # [N, D] f32
    src = np.asarray(edge_index[0], np.int64)
    dst = np.asarray(edge_index[1], np.int64)
    src = np.concatenate([src, np.arange(N, dtype=np.int64)])
    dst = np.concatenate([dst, np.arange(N, dtype=np.int64)])

    deg = np.bincount(dst, minlength=NP)  # pad nodes deg 0
    order = np.argsort(-deg, kind="stable")  # [NP]
    pos = np.empty(NP, np.int64)
    # chunk rank r -> core r%8, local j=r//8; pos = core*SHARD + j*128 + i
    for r in range(NTILES):
        nodes = order[r * 128:(r + 1) * 128]
        core, j = r % NCORES, r // NCORES
        pos[nodes] = core * SHARD + j * 128 + np.arange(128)

    srcp = pos[src]
    dstp = pos[dst]

    # group edges by dst position
    o = np.argsort(dstp, kind="stable")
    dst_sorted = dstp[o]
    src_sorted = srcp[o]
    starts = np.searchsorted(dst_sorted, np.arange(NP))
    ends = np.searchsorted(dst_sorted, np.arange(NP) + 1)
    degs_pos = ends - starts  # degree by position

    # KBAR[j] = max degree among all cores' chunks with local index j
    dp = degs_pos.reshape(NCORES, NCHUNK, 128)
    KBAR = dp.max(axis=(0, 2)).astype(np.int64)  # [NCHUNK]
    KBAR = np.maximum(KBAR, 1)
    SK = int(KBAR.sum())

    srcidx = np.zeros((NCORES, 128, SK), np.int32)
    mask = np.zeros((NCORES, 128, SK), np.float32)
    koff = np.concatenate([[0], np.cumsum(KBAR)])
    for c in range(NCORES):
        for j in range(NCHUNK):
            base = c * SHARD + j * 128
            K = int(KBAR[j])
            for p in range(128):
                s, e = starts[base + p], ends[base + p]
                d = e - s
                if d:
                    srcidx[c, p, koff[j]:koff[j] + d] = src_sorted[s:e]
                    mask[c, p, koff[j]:koff[j] + d] = 1.0

    h0p = np.zeros((NP, D), np.float32)
    h0p[pos[:N]] = h0
    bf = ml_dtypes.bfloat16
    h0pb = h0p.astype(bf)

    def wasd(W, a_s, a_d):
        return np.stack([W @ a_s, W @ a_d], axis=1).astype(bf)

    ones0 = np.zeros((128, 128), np.float32)
    ones0[0, :] = 1.0
    common = {
        "W1": W1.astype(bf), "wasd1": wasd(W1, as1, ad1),
        "W2": W2.astype(bf), "wasd2": wasd(W2, as2, ad2),
        "W3": W3.astype(bf), "wasd3": wasd(W3, as3, ad3),
        "bcol1": b1[:, None].astype(np.float32),
        "bcol2": b2[:, None].astype(np.float32),
        "bcol3": b3[:, None].astype(np.float32),
        "Wo": np.asarray(Wo).astype(bf),
        "borep": np.tile(bo[None, :], (128, 1)).astype(bf),
        "ones0": ones0.astype(bf),
        "ident": np.eye(128, dtype=np.float32).astype(bf),
    }
    in_maps = []
    for c in range(NCORES):
        m = dict(common)
        m["h0T_sh"] = np.ascontiguousarray(
            h0p[c * SHARD:(c + 1) * SHARD].T).astype(bf)
        m["srcidx"] = srcidx[c]
        m["maskin"] = mask[c].astype(bf)
        # host-side layer-0 gather: edge-major h0 rows + transposed windows
        arr = h0pb[srcidx[c]]                       # [128 d, SK, 128 f] bf16
        m["G0h"] = np.ascontiguousarray(arr).reshape(128, SK * 128)
        m["G0t"] = np.ascontiguousarray(arr.transpose(2, 1, 0)).reshape(128, SK * 128)
        in_maps.append(m)
    alphas = (float(np.asarray(p1).ravel()[0]),
              float(np.asarray(p2).ravel()[0]),
              float(np.asarray(p3).ravel()[0]))
    return KBAR, in_maps, pos, alphas


def kernel(**inputs):
    x = inputs["x"]
    edge_index = inputs["edge_index"]
    emb = inputs["emb"]
    weights = tuple(np.asarray(inputs[k], np.float32) for k in (
        "W1", "as1", "ad1", "b1", "p1", "W2", "as2", "ad2", "b2", "p2",
        "W3", "as3", "ad3", "b3", "p3", "Wo", "bo"))
    KBAR, in_maps, pos, alphas = _prepare(x, edge_index, emb, weights)

    import hashlib
    hsh = hashlib.sha256()
    hsh.update(np.asarray(edge_index).tobytes())
    hsh.update(np.asarray(emb, np.float32).tobytes())
    hsh.update(np.asarray(x).tobytes())
    for w in weights:
        hsh.update(w.tobytes())
    ck = ("gat2", hsh.hexdigest())
    if ck not in _cache:
        nc = _build_nc(KBAR, alphas)
        _cache[ck] = _Runner(nc, in_maps, NCORES)
    runner = _cache[ck]
    res = runner.results()
    full = np.concatenate([res[c]["out_sh"] for c in range(NCORES)], axis=0)  # [NP, 128]
    return full[pos[:N]].astype(np.float32)


if __name__ == "__main__":
    # quick self-test against the reference (reference runs on CPU backend)
    sys.path.insert(0, '/root/problem')
    import jax
    cpu = jax.devices("cpu")[0]
    with jax.default_device(cpu):
        import reference
        inputs = {k: np.asarray(v) for k, v in reference.setup_inputs().items()}
        exp = np.asarray(reference.reference(**{k: jax.device_put(v, cpu) for k, v in inputs.items()}))
    got = kernel(**inputs)
    err = np.abs(got - exp).max() / (np.abs(exp).max() + 1e-9)
    print("rel err:", err)


# revision 29
# speedup vs baseline: 1.4779x; 1.0083x over previous
"""3-layer GAT + linear head on 8 Trainium2 NeuronCores.

Strategy: destination-sharded edge parallelism, gather-raw-h formulation.
 - Host relabels nodes by in-degree (desc) and deals 128-node chunks
   round-robin to the 8 cores (balanced per-chunk max degrees). Each core
   owns 6272 destination slots (49 chunks of 128).
 - By linearity, out = (sum_k alpha_k h[src_k]) @ W: W is applied AFTER
   aggregation (one matmul per own chunk), so the gather table holds RAW
   rows [h_l | es_l] (129 bf16 cols) instead of transformed features and
   no replicated transform is needed.
 - Layer 0 needs no gather at all: the edge-major h0 rows are a pure
   layout transform of the input, so the host pre-builds them (G0h) plus
   transposed per-slot windows (G0t) used by PE matvecs that produce the
   per-slot es0 on device. Layers 1,2 exchange [h|es] shards with one
   AllGather each and use per-slot indirect DMAs (the only gather
   primitive this toolchain supports; ~1.1us each on the gpsimd queue).
 - Aggregation accumulates A^T = sum_k (alpha_k * h_src)^T in PSUM via
   matmul(lhsT=Gs_k, rhs=I), keeping everything feature-major so the W
   transform, per-feature bias (Act bias column) and next-layer es/ed
   matvecs need no transposes; table rows are produced with one PE
   transpose per chunk, and the final linear layer is fused into the
   layer-2 chunk epilogue (bias added via a rank-1 PE matmul).
 - Softmax: ed-add folded into the Prelu bias, Prelu instead of Lrelu so
   Prelu+Exp+Copy share one activation table (no 1.3us table reloads),
   masked denominator, alpha normalization pre-folded into the edge
   weights so the epilogue needs no rescale.
"""
import sys
sys.path.insert(0, '/opt/trn_rl_repo')
import numpy as np
import ml_dtypes

N = 50000
E = 800000
D = 128
NCORES = 8
NP = 50176           # N padded to 392 chunks of 128
SHARD = NP // NCORES   # 6272
NCHUNK = SHARD // 128  # 49
NTILES = NP // 128     # 392
NEG_SLOPE = 0.2
# Exchange split: each [h|es] AllGather is issued as 4 quarter-collectives,
# each covering a contiguous range of local chunks, so quarters fire as soon
# as their chunk rows are staged and overlap the remaining chunks' gathers.
QSPLIT = (13, 12, 12, 12)
QSTART = (0, 13, 25, 37)

_cache = {}


def _table_row(g):
    """Table row index for global node position g under the quartered
    rank-major AllGather layout."""
    g = np.asarray(g, np.int64)
    c = g // SHARD
    r = g % SHARD
    j = r // 128
    p = r % 128
    qs = np.asarray(QSTART, np.int64)
    nq = np.asarray(QSPLIT, np.int64)
    q = np.searchsorted(qs, j, side="right") - 1
    return 1024 * qs[q] + c * 128 * nq[q] + (j - qs[q]) * 128 + p


def _legalize_single_wait(nc, mybir):
    ctr = 0
    for fn in nc.m.functions:
        for bb in fn.blocks:
            insts = bb.instructions
            out = []
            changed = False
            for inst in insts:
                si = getattr(inst, 'sync_info', None) if hasattr(inst, 'sync_info') else None
                waits = list(si.on_wait) if si and si.on_wait else []
                if len(waits) > 1:
                    eng = inst.engine
                    for w in waits[:-1]:
                        ctr += 1
                        nop = mybir.InstNoOp(name=f"legwait-{ctr}", ins=[], outs=[])
                        nop.engine = eng
                        nop.sync_info = mybir.SyncInfo(on_wait=[w], on_update=[])
                        out.append(nop)
                    inst.sync_info = mybir.SyncInfo(
                        on_wait=waits[-1:], on_update=list(si.on_update or []))
                    changed = True
                out.append(inst)
            if changed:
                bb.instructions = out


def _build_nc(KBAR, alphas):
    """alphas = (p1, p2, p3) PReLU slopes baked as immediates."""
    import concourse.bass as bass
    import concourse.mybir as mybir
    from concourse.tile import TileContext

    SK = int(sum(KBAR))
    f32 = mybir.dt.float32
    bf16 = mybir.dt.bfloat16
    PRELU = mybir.ActivationFunctionType.Prelu
    EXP = mybir.ActivationFunctionType.Exp
    COPY = mybir.ActivationFunctionType.Copy

    nc = bass.Bass()
    # ---- inputs (replicated unless noted)
    # layer-0 gather pre-done on host: edge-major h0 rows, plus transposed
    # windows for the per-slot es0 matvecs (both per-core)
    G0h = nc.dram_tensor("G0h", [128, SK * 128], bf16, kind="ExternalInput")
    G0t = nc.dram_tensor("G0t", [128, SK * 128], bf16, kind="ExternalInput")
    h0T_sh = nc.dram_tensor("h0T_sh", [128, SHARD], bf16, kind="ExternalInput")  # per-core
    srcidx = nc.dram_tensor("srcidx", [128, SK], mybir.dt.int32, kind="ExternalInput")  # per-core
    maskin = nc.dram_tensor("maskin", [128, SK], bf16, kind="ExternalInput")  # per-core
    W_in, wasd_in, bcol_in = [], [], []
    for l in (1, 2, 3):
        W_in.append(nc.dram_tensor(f"W{l}", [128, 128], bf16, kind="ExternalInput"))
        wasd_in.append(nc.dram_tensor(f"wasd{l}", [128, 2], bf16, kind="ExternalInput"))
        bcol_in.append(nc.dram_tensor(f"bcol{l}", [128, 1], f32, kind="ExternalInput"))
    Wo = nc.dram_tensor("Wo", [128, 128], bf16, kind="ExternalInput")
    bo_rep = nc.dram_tensor("borep", [128, 128], bf16, kind="ExternalInput")
    ones0 = nc.dram_tensor("ones0", [128, 128], bf16, kind="ExternalInput")
    ident = nc.dram_tensor("ident", [128, 128], bf16, kind="ExternalInput")
    out_sh = nc.dram_tensor("out_sh", [SHARD, 128], f32, kind="ExternalOutput")
    # ---- internals
    Tst = nc.dram_tensor("Tst", [SHARD, 129], bf16, kind="Internal")
    TableA = nc.dram_tensor("TableA", [NP, 129], bf16, kind="Internal", addr_space="Shared")
    TableB = nc.dram_tensor("TableB", [NP, 129], bf16, kind="Internal", addr_space="Shared")
    Tables = [TableA, TableB]

    koff = np.zeros(NCHUNK + 1, np.int64)
    for j in range(NCHUNK):
        koff[j + 1] = koff[j] + KBAR[j]

    with TileContext(nc) as tc:
        with (
            tc.tile_pool(name="consts", bufs=1) as cpool,
            tc.tile_pool(name="gbuf", bufs=4) as gpool,
            tc.tile_pool(name="gtb", bufs=3) as gtpool,
            tc.tile_pool(name="gsb", bufs=3) as gspool,
            tc.tile_pool(name="sca", bufs=4) as spool,
            tc.tile_pool(name="ev", bufs=3) as epool,
            tc.tile_pool(name="psa", bufs=2, space="PSUM") as psa,
            tc.tile_pool(name="psw", bufs=2, space="PSUM") as psw,
            tc.tile_pool(name="pst", bufs=1, space="PSUM") as pst,
            tc.tile_pool(name="pse", bufs=2, space="PSUM") as pse,
        ):
            # ---- load constants
            ident_sb = cpool.tile([128, 128], bf16)
            nc.sync.dma_start(ident_sb[:], ident[:])
            ones0_sb = cpool.tile([128, 128], bf16)
            nc.sync.dma_start(ones0_sb[:], ones0[:])
            Wo_sb = cpool.tile([128, 128], bf16)
            nc.sync.dma_start(Wo_sb[:], Wo[:])
            borep_sb = cpool.tile([128, 128], bf16)
            nc.sync.dma_start(borep_sb[:], bo_rep[:])
            W_sb, wasd_sb, bcol_sb = [], [], []
            for l in range(3):
                t = cpool.tile([128, 128], bf16, tag=f"W{l}")
                nc.sync.dma_start(t[:], W_in[l][:])
                W_sb.append(t)
                t = cpool.tile([128, 2], bf16, tag=f"wasd{l}")
                nc.sync.dma_start(t[:], wasd_in[l][:])
                wasd_sb.append(t)
                t = cpool.tile([128, 1], f32, tag=f"bcol{l}")
                nc.sync.dma_start(t[:], bcol_in[l][:])
                bcol_sb.append(t)
            srcidx_sb = cpool.tile([128, SK], mybir.dt.int32)
            nc.sync.dma_start(srcidx_sb[:], srcidx[:])
            mask_sb = cpool.tile([128, SK], bf16)
            nc.sync.dma_start(mask_sb[:], maskin[:])
            h0T_all = cpool.tile([128, SHARD], bf16)
            nc.sync.dma_start(h0T_all[:], h0T_sh[:])

            # per-layer own-shard ed columns ([128, NCHUNK] f32)
            ed_sb = [cpool.tile([128, NCHUNK], f32, tag=f"ed{l}", name=f"ed{l}")
                     for l in range(3)]

            # ---- prologue: ed0 for own shard (ed0[d] = h0[d] @ (W1 ad1))
            pE = pse.tile([128, NCHUNK], f32, tag="pes")
            for j in range(NCHUNK):
                nc.tensor.matmul(pE[:, j:j + 1],
                                 lhsT=h0T_all[:, j * 128:(j + 1) * 128],
                                 rhs=wasd_sb[0][:, 1:2], start=True, stop=True)
            nc.scalar.activation(ed_sb[0][:], pE[:], COPY)

            # ---- layers
            for layer in range(3):
                for j in range(NCHUNK):
                    K = int(KBAR[j])
                    o0 = int(koff[j])
                    if layer == 0:
                        # host-pregathered edge-major h0 (+ transposed windows)
                        Gc = gpool.tile([128, K * 128], bf16, tag="Gc")
                        nc.sync.dma_start(
                            Gc[:], G0h[:, o0 * 128:(o0 + K) * 128])
                        GH = Gc[:].rearrange("p (k e) -> p k e", e=128)
                        Gt = gtpool.tile([128, K * 128], bf16, tag="Gt")
                        nc.sync.dma_start(
                            Gt[:], G0t[:, o0 * 128:(o0 + K) * 128])
                        pes = pse.tile([128, K], f32, tag="pes")
                        for k in range(K):
                            nc.tensor.matmul(
                                pes[:, k:k + 1],
                                lhsT=Gt[:, k * 128:(k + 1) * 128],
                                rhs=wasd_sb[0][:, 0:1], start=True, stop=True)
                        es_view = pes[:]
                    else:
                        Gc = gpool.tile([128, K * 129], bf16, tag="Gc")
                        G3 = Gc[:].rearrange("p (k e) -> p k e", e=129)
                        Trd = Tables[layer - 1]
                        for k in range(K):
                            nc.gpsimd.indirect_dma_start(
                                out=G3[:, k, :], out_offset=None, in_=Trd[:],
                                in_offset=bass.IndirectOffsetOnAxis(
                                    ap=srcidx_sb[:, o0 + k:o0 + k + 1], axis=0))
                        es_view = G3[:, :, 128]
                        GH = G3[:, :, 0:128]

                    # softmax over the K in-edge slots (exact, masked)
                    tL = spool.tile([128, K], f32, tag="tL")
                    nc.scalar.activation(tL[:], es_view, PRELU,
                                         bias=ed_sb[layer][:, j:j + 1], scale=1.0,
                                         alpha=NEG_SLOPE)
                    mx = spool.tile([128, 1], f32, tag="mx")
                    nc.vector.tensor_reduce(mx[:], tL[:], axis=mybir.AxisListType.X,
                                            op=mybir.AluOpType.max)
                    ngm = spool.tile([128, 1], f32, tag="ngm")
                    nc.vector.tensor_scalar(out=ngm[:], in0=mx[:], scalar1=-1.0,
                                            scalar2=None, op0=mybir.AluOpType.mult)
                    wE = spool.tile([128, K], bf16, tag="wE")
                    nc.scalar.activation(wE[:], tL[:], EXP, bias=ngm[:, 0:1], scale=1.0)
                    w2 = spool.tile([128, K], bf16, tag="w2")
                    nc.vector.tensor_tensor(out=w2[:], in0=wE[:],
                                            in1=mask_sb[:, o0:o0 + K],
                                            op=mybir.AluOpType.mult)
                    zz = spool.tile([128, 1], f32, tag="zz")
                    nc.vector.tensor_reduce(zz[:], w2[:], axis=mybir.AxisListType.X,
                                            op=mybir.AluOpType.add)
                    zc = spool.tile([128, 1], f32, tag="zc")
                    nc.vector.tensor_scalar(out=zc[:], in0=zz[:], scalar1=1e-30,
                                            scalar2=None, op0=mybir.AluOpType.max)
                    zi = spool.tile([128, 1], f32, tag="zi")
                    nc.vector.reciprocal(zi[:], zc[:])
                    al = spool.tile([128, K], bf16, tag="al")
                    nc.vector.tensor_scalar(out=al[:], in0=w2[:], scalar1=zi[:, 0:1],
                                            scalar2=None, op0=mybir.AluOpType.mult)
                    Gs = gspool.tile([128, K * 128], bf16, tag="Gs")
                    Gs3 = Gs[:].rearrange("p (k e) -> p k e", e=128)
                    nc.vector.tensor_tensor(
                        out=Gs3[:, :, :], in0=GH,
                        in1=al[:].unsqueeze(2).broadcast_to((128, K, 128)),
                        op=mybir.AluOpType.mult)

                    # A^T = sum_k (alpha_k h_src)^T accumulated in PSUM
                    pa = psa.tile([128, 128], f32, tag="pa")
                    for k in range(K):
                        nc.tensor.matmul(pa[:], lhsT=Gs3[:, k, :], rhs=ident_sb[:],
                                         start=(k == 0), stop=(k == K - 1))
                    At = epool.tile([128, 128], bf16, tag="At")
                    nc.scalar.activation(At[:], pa[:], COPY)
                    # h_next^T = prelu(W^T A^T + b)
                    ph = psw.tile([128, 128], f32, tag="ph")
                    nc.tensor.matmul(ph[:], lhsT=W_sb[layer][:], rhs=At[:],
                                     start=True, stop=True)
                    hT = epool.tile([128, 128], bf16, tag="hT")
                    nc.scalar.activation(hT[:], ph[:], PRELU,
                                         bias=bcol_sb[layer][:, 0:1], scale=1.0,
                                         alpha=float(alphas[layer]))

                    if layer < 2:
                        # es/ed for next layer + row-major [h|es] table rows
                        pe2 = pse.tile([128, 2], f32, tag="pes")
                        nc.tensor.matmul(pe2[:], lhsT=hT[:], rhs=wasd_sb[layer + 1][:],
                                         start=True, stop=True)
                        pt = pst.tile([128, 128], bf16, tag="pt")
                        nc.tensor.transpose(pt[:], hT[:], ident_sb[:])
                        stage = epool.tile([128, 129], bf16, tag="stage")
                        nc.scalar.activation(stage[:, 0:128], pt[:], COPY)
                        nc.scalar.activation(stage[:, 128:129], pe2[:, 0:1], COPY)
                        nc.vector.tensor_scalar(out=ed_sb[layer + 1][:, j:j + 1],
                                                in0=pe2[:, 1:2], scalar1=0.0,
                                                scalar2=None, op0=mybir.AluOpType.add)
                        nc.sync.dma_start(Tst[j * 128:(j + 1) * 128, :], stage[:])
                        # fire the quarter-AllGather as soon as its chunk
                        # range is fully staged; it overlaps later chunks
                        for q in range(4):
                            if j == QSTART[q] + QSPLIT[q] - 1:
                                q0, nq = QSTART[q], QSPLIT[q]
                                nc.gpsimd.collective_compute(
                                    "AllGather", mybir.AluOpType.bypass,
                                    ins=[Tst[q0 * 128:(q0 + nq) * 128, :]],
                                    outs=[Tables[layer][1024 * q0:1024 * (q0 + nq), :]],
                                    replica_groups=[list(range(NCORES))],
                                )
                    else:
                        # final linear fused into the layer-2 epilogue
                        po = psw.tile([128, 128], f32, tag="ph")
                        nc.tensor.matmul(po[:], lhsT=hT[:], rhs=Wo_sb[:],
                                         start=True, stop=False)
                        nc.tensor.matmul(po[:], lhsT=ones0_sb[:], rhs=borep_sb[:],
                                         start=False, stop=True)
                        oo = epool.tile([128, 128], f32, tag="oo")
                        nc.scalar.activation(oo[:], po[:], COPY)
                        nc.sync.dma_start(out_sh[j * 128:(j + 1) * 128, :], oo[:])



    _legalize_single_wait(nc, mybir)
    return nc


class _Runner:
    def __init__(self, nc, in_maps, n_cores):
        import jax
        import concourse.mybir as mybir
        from concourse.bass2jax import (_bass_exec_p, partition_id_tensor,
                                        install_neuronx_cc_hook)
        from jax.sharding import Mesh, PartitionSpec
        from jax.experimental.shard_map import shard_map
        install_neuronx_cc_hook()
        self.jax = jax
        self.n_cores = n_cores
        in_names, out_names, out_avals, zero_outs = [], [], [], []
        partition_name = nc.partition_id_tensor.name if nc.partition_id_tensor else None
        for alloc in nc.m.functions[0].allocations:
            if not isinstance(alloc, mybir.MemoryLocationSet):
                continue
            name = alloc.memorylocations[0].name
            if alloc.kind == "ExternalInput":
                if name != partition_name:
                    in_names.append(name)
            elif alloc.kind == "ExternalOutput":
                shape = tuple(alloc.tensor_shape)
                dtype = mybir.dt.np(alloc.dtype)
                out_names.append(name)
                out_avals.append(jax.core.ShapedArray(shape, dtype))
                zero_outs.append(np.zeros(shape, dtype))
        n_params = len(in_names)
        self.out_names, self.out_avals = out_names, out_avals
        all_in = list(in_names) + list(out_names)
        if partition_name is not None:
            all_in.append(partition_name)

        def _body(*args):
            operands = list(args)
            if partition_name is not None:
                operands.append(partition_id_tensor())
            outs = _bass_exec_p.bind(
                *operands, out_avals=tuple(out_avals), in_names=tuple(all_in),
                out_names=tuple(out_names), lowering_input_output_aliases=(),
                sim_require_finite=False, sim_require_nnan=False, nc=nc)
            return tuple(outs)

        devices = jax.devices()[:n_cores]
        mesh = Mesh(np.asarray(devices), ("core",))
        self.fn = jax.jit(
            shard_map(_body, mesh=mesh,
                      in_specs=(PartitionSpec("core"),) * (n_params + len(out_names)),
                      out_specs=(PartitionSpec("core"),) * len(out_names),
                      check_rep=False),
            keep_unused=True)
        per_core = [[np.asarray(m[nm]) for nm in in_names] for m in in_maps]
        concat_in = [np.concatenate([per_core[c][i] for c in range(n_cores)], axis=0)
                     for i in range(n_params)]
        concat_zeros = [np.zeros((n_cores * z.shape[0], *z.shape[1:]), z.dtype)
                        for z in zero_outs]
        sh = jax.sharding.NamedSharding(mesh, PartitionSpec("core"))
        self.dev_args = [jax.device_put(a, sh) for a in concat_in + concat_zeros]

    def run_raw(self):
        return self.fn(*self.dev_args)

    def results(self):
        outs = self.run_raw()
        self.jax.block_until_ready(outs)
        return [
            {nm: np.asarray(outs[i]).reshape(self.n_cores, *self.out_avals[i].shape)[c]
             for i, nm in enumerate(self.out_names)}
            for c in range(self.n_cores)]


def _prepare(x, edge_index, emb, weights):
    """Host-side: relabel, chunk, schedule, build per-core inputs."""
    (W1, as1, ad1, b1, p1, W2, as2, ad2, b2, p2,
     W3, as3, ad3, b3, p3, Wo, bo) = weights
    h0 = np.asarray(emb)[np.asarray(x)]  # [N, D] f32
    src = np.asarray(edge_index[0], np.int64)
    dst = np.asarray(edge_index[1], np.int64)
    src = np.concatenate([src, np.arange(N, dtype=np.int64)])
    dst = np.concatenate([dst, np.arange(N, dtype=np.int64)])

    deg = np.bincount(dst, minlength=NP)  # pad nodes deg 0
    order = np.argsort(-deg, kind="stable")  # [NP]
    pos = np.empty(NP, np.int64)
    # chunk rank r -> core r%8, local j=r//8; pos = core*SHARD + j*128 + i
    for r in range(NTILES):
        nodes = order[r * 128:(r + 1) * 128]
        core, j = r % NCORES, r // NCORES
        pos[nodes] = core * SHARD + j * 128 + np.arange(128)

    srcp = pos[src]
    dstp = pos[dst]

    # group edges by dst position
    o = np.argsort(dstp, kind="stable")
    dst_sorted = dstp[o]
    src_sorted = srcp[o]
    starts = np.searchsorted(dst_sorted, np.arange(NP))
    ends = np.searchsorted(dst_sorted, np.arange(NP) + 1)
    degs_pos = ends - starts  # degree by position

    # KBAR[j] = max degree among all cores' chunks with local index j
    dp = degs_pos.reshape(NCORES, NCHUNK, 128)
    KBAR = dp.max(axis=(0, 2)).astype(np.int64)  # [NCHUNK]
    KBAR = np.maximum(KBAR, 1)
    SK = int(KBAR.sum())

    src_tab = _table_row(src_sorted)  # table rows under quartered layout
    srcidx = np.zeros((NCORES, 128, SK), np.int32)   # table rows (device gathers)
    srcpos = np.zeros((NCORES, 128, SK), np.int32)   # global positions (host G0)
    mask = np.zeros((NCORES, 128, SK), np.float32)
    koff = np.concatenate([[0], np.cumsum(KBAR)])
    for c in range(NCORES):
        for j in range(NCHUNK):
            base = c * SHARD + j * 128
            K = int(KBAR[j])
            for p in range(128):
                s, e = starts[base + p], ends[base + p]
                d = e - s
                if d:
                    srcidx[c, p, koff[j]:koff[j] + d] = src_tab[s:e]
                    srcpos[c, p, koff[j]:koff[j] + d] = src_sorted[s:e]
                    mask[c, p, koff[j]:koff[j] + d] = 1.0

    h0p = np.zeros((NP, D), np.float32)
    h0p[pos[:N]] = h0
    bf = ml_dtypes.bfloat16
    h0pb = h0p.astype(bf)

    def wasd(W, a_s, a_d):
        return np.stack([W @ a_s, W @ a_d], axis=1).astype(bf)

    ones0 = np.zeros((128, 128), np.float32)
    ones0[0, :] = 1.0
    common = {
        "W1": W1.astype(bf), "wasd1": wasd(W1, as1, ad1),
        "W2": W2.astype(bf), "wasd2": wasd(W2, as2, ad2),
        "W3": W3.astype(bf), "wasd3": wasd(W3, as3, ad3),
        "bcol1": b1[:, None].astype(np.float32),
        "bcol2": b2[:, None].astype(np.float32),
        "bcol3": b3[:, None].astype(np.float32),
        "Wo": np.asarray(Wo).astype(bf),
        "borep": np.tile(bo[None, :], (128, 1)).astype(bf),
        "ones0": ones0.astype(bf),
        "ident": np.eye(128, dtype=np.float32).astype(bf),
    }
    in_maps = []
    for c in range(NCORES):
        m = dict(common)
        m["h0T_sh"] = np.ascontiguousarray(
            h0p[c * SHARD:(c + 1) * SHARD].T).astype(bf)
        m["srcidx"] = srcidx[c]
        m["maskin"] = mask[c].astype(bf)
        # host-side layer-0 gather: edge-major h0 rows + transposed windows
        arr = h0pb[srcpos[c]]                       # [128 d, SK, 128 f] bf16
        m["G0h"] = np.ascontiguousarray(arr).reshape(128, SK * 128)
        m["G0t"] = np.ascontiguousarray(arr.transpose(2, 1, 0)).reshape(128, SK * 128)
        in_maps.append(m)
    alphas = (float(np.asarray(p1).ravel()[0]),
              float(np.asarray(p2).ravel()[0]),
              float(np.asarray(p3).ravel()[0]))
    return KBAR, in_maps, pos, alphas


def kernel(**inputs):
    x = inputs["x"]
    edge_index = inputs["edge_index"]
    emb = inputs["emb"]
    weights = tuple(np.asarray(inputs[k], np.float32) for k in (
        "W1", "as1", "ad1", "b1", "p1", "W2", "as2", "ad2", "b2", "p2",
        "W3", "as3", "ad3", "b3", "p3", "Wo", "bo"))
    KBAR, in_maps, pos, alphas = _prepare(x, edge_index, emb, weights)

    import hashlib
    hsh = hashlib.sha256()
    hsh.update(np.asarray(edge_index).tobytes())
    hsh.update(np.asarray(emb, np.float32).tobytes())
    hsh.update(np.asarray(x).tobytes())
    for w in weights:
        hsh.update(w.tobytes())
    ck = ("gat2", hsh.hexdigest())
    if ck not in _cache:
        nc = _build_nc(KBAR, alphas)
        _cache[ck] = _Runner(nc, in_maps, NCORES)
    runner = _cache[ck]
    res = runner.results()
    full = np.concatenate([res[c]["out_sh"] for c in range(NCORES)], axis=0)  # [NP, 128]
    return full[pos[:N]].astype(np.float32)


if __name__ == "__main__":
    # quick self-test against the reference (reference runs on CPU backend)
    sys.path.insert(0, '/root/problem')
    import jax
    cpu = jax.devices("cpu")[0]
    with jax.default_device(cpu):
        import reference
        inputs = {k: np.asarray(v) for k, v in reference.setup_inputs().items()}
        exp = np.asarray(reference.reference(**{k: jax.device_put(v, cpu) for k, v in inputs.items()}))
    got = kernel(**inputs)
    err = np.abs(got - exp).max() / (np.abs(exp).max() + 1e-9)
    print("rel err:", err)
